# revision 3
# baseline (speedup 1.0000x reference)
"""Causal single-head attention (b=4, s=2048, d=1024) on 8 trn2 NeuronCores.

Sharding: data-parallel over batch (4) x 2-way query split per batch.
Core c = 2*b + h handles batch b and query tiles {2j+h : j=0..7} (128-row
tiles). Every core computes full K^T / V for its batch (duplicated across
the pair), Q^T only for its own 8 query tiles. Causality is handled by
rounding each slot's key range up to (2j+2) tiles and adding a per-core
additive mask (data, not program) over the last 256 columns, keeping the
SPMD program identical across cores.

All matmuls run in fp16 (1 cyc/row on PE, fp32 PSUM accumulation);
softmax runs in fp32 on ACT/DVE.
"""

import contextlib
import os
import sys
import types

import numpy as np

P = 128
SEQ = 2048
D = 1024
NB = 4
NSLOT = 8          # query tiles per core
IT = D // P        # 8 contraction tiles (d_in)
OT = D // P        # 8 output tiles (d_out)
ST = SEQ // P      # 16 seq tiles
MASK_NEG = -30000.0
SCALE = 1.0 / 32.0  # 1/sqrt(d_out)

_PROG_CACHE = {}


def _install_ntff_hook():
    """Register the NTFF profile hook this image's antenv lacks (best effort)."""
    try:
        import antenv.axon_hooks  # noqa: F401
        return
    except ImportError:
        pass
    try:
        import trn_agent_boot.trn_boot as tb
        hook = tb._ntff_profile_via_ctypes('/opt/axon/libaxon_pjrt.so')
        mod = types.ModuleType('antenv.axon_hooks')
        mod._hook = hook
        mod.get_axon_ntff_profile_hook = lambda: mod._hook

        def _set(h):
            mod._hook = h
        mod.set_axon_ntff_profile_hook = _set
        sys.modules['antenv.axon_hooks'] = mod
    except Exception:
        pass


def build_program():
    """Build + compile the single SPMD Bass program (cached)."""
    if "nc" in _PROG_CACHE:
        return _PROG_CACHE["nc"]

    from contextlib import ExitStack

    import concourse.bass as bass  # noqa: F401
    import concourse.mybir as mybir
    from concourse import bacc
    from concourse.masks import make_identity
    from concourse.tile import TileContext

    f32 = mybir.dt.float32
    f16 = mybir.dt.float16

    nc = bacc.Bacc("TRN2", target_bir_lowering=False, debug=False, num_devices=8)

    xkv_d = nc.dram_tensor("xkv", [D, SEQ], f16, kind="ExternalInput").ap()
    xq_d = nc.dram_tensor("xq", [D, NSLOT * P], f16, kind="ExternalInput").ap()
    wq_d = nc.dram_tensor("wq", [D, D], f16, kind="ExternalInput").ap()
    wk_d = nc.dram_tensor("wk", [D, D], f16, kind="ExternalInput").ap()
    wv_d = nc.dram_tensor("wv", [D, D], f16, kind="ExternalInput").ap()
    mask_d = nc.dram_tensor("mask", [P, 2 * P], f32, kind="ExternalInput").ap()
    out_d = nc.dram_tensor("out", [NSLOT * P, D], f32, kind="ExternalOutput").ap()

    with TileContext(nc) as tc, ExitStack() as ctx:
        const = ctx.enter_context(tc.tile_pool(name="const", bufs=1))
        persist = ctx.enter_context(tc.tile_pool(name="persist", bufs=1))
        # wq/wk/wv/xq live here during QKV build; O staging tiles reuse the
        # slots afterwards (same tag -> shared slots).
        big = ctx.enter_context(tc.tile_pool(name="big", bufs=4))
        ppool = ctx.enter_context(tc.tile_pool(name="ppool", bufs=2))
        ptpool = ctx.enter_context(tc.tile_pool(name="ptpool", bufs=18))
        scal = ctx.enter_context(tc.tile_pool(name="scal", bufs=16))
        work = ctx.enter_context(tc.tile_pool(name="work", bufs=6, space="PSUM"))
        opsum = ctx.enter_context(tc.tile_pool(name="opsum", bufs=1, space="PSUM"))

        ident = const.tile([P, P], f16, tag="ident")
        make_identity(nc, ident[:])
        mask_sb = const.tile([P, 2 * P], f32, tag="mask")
        nc.sync.dma_start(out=mask_sb[:], in_=mask_d)

        xkv_sb = persist.tile([P, IT, SEQ], f16, tag="xkv")
        xkv_t = xkv_d.rearrange("(i p) s -> i p s", p=P)
        for i in range(IT):
            nc.sync.dma_start(out=xkv_sb[:, i], in_=xkv_t[i])

        w_sb = {}
        for name, dram in (("wk", wk_d), ("wq", wq_d), ("wv", wv_d)):
            t = big.tile([P, IT, D], f16, tag="big")
            dt_ = dram.rearrange("(i p) o -> i p o", p=P)
            for i in range(IT):
                nc.sync.dma_start(out=t[:, i], in_=dt_[i])
            w_sb[name] = t
        xq_sb = big.tile([P, IT, NSLOT * P], f16, tag="big")
        xq_t = xq_d.rearrange("(i p) s -> i p s", p=P)
        for i in range(IT):
            nc.sync.dma_start(out=xq_sb[:, i], in_=xq_t[i])

        kT_sb = persist.tile([P, OT, SEQ], f16, tag="kT")
        qT_sb = persist.tile([P, OT, NSLOT * P], f16, tag="qT")
        v_sb = persist.tile([P, ST, D], f16, tag="v")

        # ---- K^T build: kT[o, s] = sum_i wk[i, o] * x[s, i] ----
        for o in range(OT):
            chunks = [work.tile([P, 512], f32, tag="wk", name=f"kch{o}_{c}") for c in range(4)]
            for i in range(IT):
                lhsT = w_sb["wk"][:, i, o * P:(o + 1) * P]
                for c in range(4):
                    nc.tensor.matmul(
                        chunks[c][:],
                        lhsT=lhsT,
                        rhs=xkv_sb[:, i, c * 512:(c + 1) * 512],
                        start=(i == 0),
                        stop=(i == IT - 1),
                    )
            for c in range(4):
                nc.vector.tensor_copy(
                    out=kT_sb[:, o, c * 512:(c + 1) * 512], in_=chunks[c][:]
                )

        # ---- Q^T build (xq pre-scaled by 1/32 on host) ----
        for o in range(OT):
            chunks = [work.tile([P, 512], f32, tag="wk", name=f"qch{o}_{c}") for c in range(2)]
            for i in range(IT):
                lhsT = w_sb["wq"][:, i, o * P:(o + 1) * P]
                for c in range(2):
                    nc.tensor.matmul(
                        chunks[c][:],
                        lhsT=lhsT,
                        rhs=xq_sb[:, i, c * 512:(c + 1) * 512],
                        start=(i == 0),
                        stop=(i == IT - 1),
                    )
            for c in range(2):
                nc.vector.tensor_copy(
                    out=qT_sb[:, o, c * 512:(c + 1) * 512], in_=chunks[c][:]
                )

        # ---- per-slot: V tiles, S, softmax, P^T, P^T @ V ----
        for j in range(NSLOT):
            # V build for seq tiles 2j, 2j+1: v[s, o] = sum_i x[s, i] wv[i, o]
            for st in (2 * j, 2 * j + 1):
                vch = [work.tile([P, 512], f32, tag="wk", name=f"vch{st}_{c}") for c in range(2)]
                for i in range(IT):
                    lhsT = xkv_sb[:, i, st * P:(st + 1) * P]
                    for c in range(2):
                        nc.tensor.matmul(
                            vch[c][:],
                            lhsT=lhsT,
                            rhs=w_sb["wv"][:, i, c * 512:(c + 1) * 512],
                            start=(i == 0),
                            stop=(i == IT - 1),
                        )
                for c in range(2):
                    nc.scalar.copy(
                        out=v_sb[:, st, c * 512:(c + 1) * 512], in_=vch[c][:]
                    )

            L = 2 * j + 2          # key tiles for this slot
            cols = L * P
            sizes = []
            off = 0
            while off < cols:
                sizes.append(min(512, cols - off))
                off += sizes[-1]
            nch = len(sizes)

            # S = Q^T.T @ K^T (contraction over d_out), chunked into PSUM
            sch = [work.tile([P, 512], f32, tag="wk", name=f"sch{j}_{c}") for c in range(nch)]
            for o in range(OT):
                lhsT = qT_sb[:, o, j * P:(j + 1) * P]
                for c in range(nch):
                    nc.tensor.matmul(
                        sch[c][:, :sizes[c]],
                        lhsT=lhsT,
                        rhs=kT_sb[:, o, c * 512:c * 512 + sizes[c]],
                        start=(o == 0),
                        stop=(o == OT - 1),
                    )

            # additive causal mask on the last 256 columns (in PSUM)
            mstart = cols - 256
            moff = 0
            while moff < 256:
                c = (mstart + moff) // 512
                within = (mstart + moff) - c * 512
                mlen = min(256 - moff, sizes[c] - within)
                nc.vector.tensor_tensor(
                    sch[c][:, within:within + mlen],
                    sch[c][:, within:within + mlen],
                    mask_sb[:, moff:moff + mlen],
                    mybir.AluOpType.add,
                )
                moff += mlen

            # row max over all chunks -> negated bias
            mx = scal.tile([P, 4], f32, tag="mx")
            for c in range(nch):
                nc.vector.reduce_max(
                    mx[:, c:c + 1], sch[c][:, :sizes[c]], axis=mybir.AxisListType.X
                )
            negm = scal.tile([P, 1], f32, tag="negm")
            nc.vector.reduce_max(
                negm[:], mx[:, :nch], axis=mybir.AxisListType.X, negate=True
            )

            # P = exp(S - max), fp16, with per-chunk row sums accumulated
            p_sb = ppool.tile([P, cols], f16, tag="p")
            ell = scal.tile([P, 4], f32, tag="ell")
            for c in range(nch):
                nc.scalar.activation(
                    p_sb[:, c * 512:c * 512 + sizes[c]],
                    sch[c][:, :sizes[c]],
                    mybir.ActivationFunctionType.Exp,
                    bias=negm[:],
                    scale=1.0,
                    accum_out=ell[:, c:c + 1],
                )
            ells = scal.tile([P, 1], f32, tag="ells")
            nc.vector.reduce_sum(
                ells[:], ell[:, :nch], axis=mybir.AxisListType.X
            )
            rinv = scal.tile([P, 1], f32, tag="rinv")
            nc.vector.reciprocal(rinv[:], ells[:])

            # transpose P tile-by-tile, then O = P^T.T @ V accumulation
            pts = []
            for kt in range(L):
                pt_ps = work.tile([P, P], f16, tag="wk")
                nc.tensor.transpose(
                    pt_ps[:], p_sb[:, kt * P:(kt + 1) * P], ident[:]
                )
                pt_sb = ptpool.tile([P, P], f16, tag="pt")
                nc.vector.tensor_copy(out=pt_sb[:], in_=pt_ps[:])
                pts.append(pt_sb)

            o_ps = opsum.tile([P, D], f32, tag="o")
            for kt in range(L):
                for c in range(2):
                    nc.tensor.matmul(
                        o_ps[:, c * 512:(c + 1) * 512],
                        lhsT=pts[kt][:],
                        rhs=v_sb[:, kt, c * 512:(c + 1) * 512],
                        start=(kt == 0),
                        stop=(kt == L - 1),
                    )

            o_sb = big.tile([P, D], f32, tag="big")
            nc.scalar.mul(o_sb[:], o_ps[:], rinv[:])
            nc.sync.dma_start(out=out_d[j * P:(j + 1) * P, :], in_=o_sb[:])

    nc.compile()
    _PROG_CACHE["nc"] = nc
    return nc


def make_in_maps(x, Wq, Wk, Wv):
    """Host-side sharding: returns per-core input dicts (core c = 2*b + h)."""
    x = np.asarray(x, dtype=np.float32)
    wq16 = np.asarray(Wq, dtype=np.float32).astype(np.float16)
    wk16 = np.asarray(Wk, dtype=np.float32).astype(np.float16)
    wv16 = np.asarray(Wv, dtype=np.float32).astype(np.float16)

    tri = np.where(
        np.arange(P)[None, :] <= np.arange(P)[:, None], 0.0, MASK_NEG
    ).astype(np.float32)
    masks = []
    for h in range(2):
        m = np.zeros((P, 2 * P), dtype=np.float32)
        if h == 0:
            m[:, :P] = tri
            m[:, P:] = MASK_NEG
        else:
            m[:, :P] = 0.0
            m[:, P:] = tri
        masks.append(m)

    in_maps = []
    for b in range(NB):
        xt = np.ascontiguousarray(x[b].T)          # [D, SEQ] f32
        xt16 = xt.astype(np.float16)
        for h in range(2):
            qcols = np.concatenate(
                [np.arange((2 * j + h) * P, (2 * j + h + 1) * P) for j in range(NSLOT)]
            )
            xq = np.ascontiguousarray((xt[:, qcols] * SCALE).astype(np.float16))
            in_maps.append({
                "xkv": xt16,
                "xq": xq,
                "wq": wq16,
                "wk": wk16,
                "wv": wv16,
                "mask": masks[h],
            })
    return in_maps


def assemble_output(results):
    """Gather per-core [NSLOT*P, D] outputs into the full [NB, SEQ, D]."""
    out = np.empty((NB, SEQ, D), dtype=np.float32)
    for b in range(NB):
        for h in range(2):
            r = results[2 * b + h]["out"]
            for j in range(NSLOT):
                g = 2 * j + h
                out[b, g * P:(g + 1) * P, :] = r[j * P:(j + 1) * P, :]
    return out


def run(inputs, trace=False, tmpdir=None):
    """Build, run on 8 cores, gather. Returns (output, BassKernelResults)."""
    _install_ntff_hook()
    from concourse.bass_utils import run_bass_kernel_spmd

    nc = build_program()
    in_maps = make_in_maps(
        inputs["x"], inputs["Wq"], inputs["Wk"], inputs["Wv"]
    )
    kw = {}
    if trace:
        kw["trace"] = True
        if tmpdir is not None:
            kw["tmpdir"] = tmpdir
    res = run_bass_kernel_spmd(nc, in_maps, list(range(8)), **kw)
    return assemble_output(res.results), res


def kernel(**inputs):
    out, _ = run(inputs, trace=False)
    return out


# revision 4
# speedup vs baseline: 1.1490x; 1.1490x over previous
"""Causal single-head attention (b=4, s=2048, d=1024) on 8 trn2 NeuronCores.

Sharding: data-parallel over batch (4) x 2-way query split per batch.
Core c = 2*b + h handles batch b and query tiles {2j+h : j=0..7} (128-row
tiles). Every core computes full K^T / V for its batch (duplicated across
the pair), Q^T only for its own 8 query tiles. Causality is handled by
rounding each slot's key range up to (2j+2) tiles and adding a per-core
additive mask (data, not program) over the last 256 columns, keeping the
SPMD program identical across cores.

All matmuls run in fp16 (1 cyc/row on PE, fp32 PSUM accumulation);
softmax runs in fp32 on ACT/DVE. The attention slots are software-
pipelined: while PE computes V/S for slot j+1, ACT/DVE run softmax(j);
PE then transposes P(j) and accumulates O(j) without stalling.
"""

import sys
import types

import numpy as np

P = 128
SEQ = 2048
D = 1024
NB = 4
NSLOT = 8          # query tiles per core
IT = D // P        # 8 contraction tiles (d_in)
OT = D // P        # 8 output tiles (d_out)
ST = SEQ // P      # 16 seq tiles
MASK_NEG = -30000.0
SCALE = 1.0 / 32.0  # 1/sqrt(d_out)

_PROG_CACHE = {}


def _install_ntff_hook():
    """Register the NTFF profile hook this image's antenv lacks (best effort)."""
    try:
        import antenv.axon_hooks  # noqa: F401
        return
    except ImportError:
        pass
    try:
        import trn_agent_boot.trn_boot as tb
        hook = tb._ntff_profile_via_ctypes('/opt/axon/libaxon_pjrt.so')
        mod = types.ModuleType('antenv.axon_hooks')
        mod._hook = hook
        mod.get_axon_ntff_profile_hook = lambda: mod._hook

        def _set(h):
            mod._hook = h
        mod.set_axon_ntff_profile_hook = _set
        sys.modules['antenv.axon_hooks'] = mod
    except Exception:
        pass


def _chunk_sizes(cols):
    sizes = []
    off = 0
    while off < cols:
        sizes.append(min(512, cols - off))
        off += sizes[-1]
    return sizes


def build_program():
    """Build + compile the single SPMD Bass program (cached)."""
    if "nc" in _PROG_CACHE:
        return _PROG_CACHE["nc"]

    from contextlib import ExitStack

    import concourse.mybir as mybir
    from concourse import bacc
    from concourse.masks import make_identity
    from concourse.tile import TileContext

    f32 = mybir.dt.float32
    f16 = mybir.dt.float16
    ADD = mybir.AluOpType.add
    AXX = mybir.AxisListType.X
    EXP = mybir.ActivationFunctionType.Exp

    nc = bacc.Bacc("TRN2", target_bir_lowering=False, debug=False, num_devices=8)

    xkv_d = nc.dram_tensor("xkv", [D, SEQ], f16, kind="ExternalInput").ap()
    xq_d = nc.dram_tensor("xq", [D, NSLOT * P], f16, kind="ExternalInput").ap()
    wq_d = nc.dram_tensor("wq", [D, D], f16, kind="ExternalInput").ap()
    wk_d = nc.dram_tensor("wk", [D, D], f16, kind="ExternalInput").ap()
    wv_d = nc.dram_tensor("wv", [D, D], f16, kind="ExternalInput").ap()
    mask_d = nc.dram_tensor("mask", [P, 2 * P], f32, kind="ExternalInput").ap()
    out_d = nc.dram_tensor("out", [NSLOT * P, D], f32, kind="ExternalOutput").ap()

    with TileContext(nc) as tc, ExitStack() as ctx:
        const = ctx.enter_context(tc.tile_pool(name="const", bufs=1))
        persist = ctx.enter_context(tc.tile_pool(name="persist", bufs=1))
        big = ctx.enter_context(tc.tile_pool(name="big", bufs=4))
        s32p = ctx.enter_context(tc.tile_pool(name="s32p", bufs=2))
        ppool = ctx.enter_context(tc.tile_pool(name="ppool", bufs=2))
        ptpool = ctx.enter_context(tc.tile_pool(name="ptpool", bufs=18))
        scal = ctx.enter_context(tc.tile_pool(name="scal", bufs=16))
        work = ctx.enter_context(tc.tile_pool(name="work", bufs=6, space="PSUM"))
        opsum = ctx.enter_context(tc.tile_pool(name="opsum", bufs=1, space="PSUM"))

        mask_sb = const.tile([P, 2 * P], f32, tag="mask")
        nc.sync.dma_start(out=mask_sb[:], in_=mask_d)
        ident = const.tile([P, P], f16, tag="ident")
        make_identity(nc, ident[:])

        # ---- input DMAs, ordered so K-build can start ASAP ----
        wk_sb = big.tile([P, IT, D], f16, tag="big", name="wk_sb")
        wq_sb = big.tile([P, IT, D], f16, tag="big", name="wq_sb")
        wv_sb = big.tile([P, IT, D], f16, tag="big", name="wv_sb")
        xq_sb = big.tile([P, IT, NSLOT * P], f16, tag="big", name="xq_sb")
        xkv_sb = persist.tile([P, IT, SEQ], f16, tag="xkv")

        xkv_t = xkv_d.rearrange("(i p) s -> i p s", p=P)
        wk_t = wk_d.rearrange("(i p) o -> i p o", p=P)
        for i in range(IT):
            nc.sync.dma_start(out=wk_sb[:, i], in_=wk_t[i])
            nc.sync.dma_start(out=xkv_sb[:, i], in_=xkv_t[i])
        wq_t = wq_d.rearrange("(i p) o -> i p o", p=P)
        xq_t = xq_d.rearrange("(i p) s -> i p s", p=P)
        for i in range(IT):
            nc.sync.dma_start(out=wq_sb[:, i], in_=wq_t[i])
            nc.sync.dma_start(out=xq_sb[:, i], in_=xq_t[i])
        wv_t = wv_d.rearrange("(i p) o -> i p o", p=P)
        for i in range(IT):
            nc.sync.dma_start(out=wv_sb[:, i], in_=wv_t[i])

        kT_sb = persist.tile([P, OT, SEQ], f16, tag="kT")
        qT_sb = persist.tile([P, OT, NSLOT * P], f16, tag="qT")
        v_sb = persist.tile([P, ST, D], f16, tag="v")

        # ---- K^T build: kT[o, s] = sum_i wk[i, o] * x[s, i] ----
        for o in range(OT):
            chunks = [work.tile([P, 512], f32, tag="wk", name=f"kch{o}_{c}")
                      for c in range(4)]
            for i in range(IT):
                lhsT = wk_sb[:, i, o * P:(o + 1) * P]
                for c in range(4):
                    nc.tensor.matmul(
                        chunks[c][:],
                        lhsT=lhsT,
                        rhs=xkv_sb[:, i, c * 512:(c + 1) * 512],
                        start=(i == 0),
                        stop=(i == IT - 1),
                    )
            for c in range(4):
                nc.vector.tensor_copy(
                    out=kT_sb[:, o, c * 512:(c + 1) * 512], in_=chunks[c][:]
                )

        # ---- Q^T build (xq pre-scaled by 1/32 on host) ----
        for o in range(OT):
            chunks = [work.tile([P, 512], f32, tag="wk", name=f"qch{o}_{c}")
                      for c in range(2)]
            for i in range(IT):
                lhsT = wq_sb[:, i, o * P:(o + 1) * P]
                for c in range(2):
                    nc.tensor.matmul(
                        chunks[c][:],
                        lhsT=lhsT,
                        rhs=xq_sb[:, i, c * 512:(c + 1) * 512],
                        start=(i == 0),
                        stop=(i == IT - 1),
                    )
            for c in range(2):
                nc.vector.tensor_copy(
                    out=qT_sb[:, o, c * 512:(c + 1) * 512], in_=chunks[c][:]
                )

        def emit_v_build(jj):
            """V rows for seq tiles 2jj, 2jj+1: v[s,o] = sum_i x[s,i] wv[i,o]."""
            for st in (2 * jj, 2 * jj + 1):
                vch = [work.tile([P, 512], f32, tag="wk", name=f"vch{st}_{c}")
                       for c in range(2)]
                for i in range(IT):
                    lhsT = xkv_sb[:, i, st * P:(st + 1) * P]
                    for c in range(2):
                        nc.tensor.matmul(
                            vch[c][:],
                            lhsT=lhsT,
                            rhs=wv_sb[:, i, c * 512:(c + 1) * 512],
                            start=(i == 0),
                            stop=(i == IT - 1),
                        )
                for c in range(2):
                    nc.scalar.copy(
                        out=v_sb[:, st, c * 512:(c + 1) * 512], in_=vch[c][:]
                    )

        def emit_s(jj):
            """S(jj) = Q^T.T @ K^T into PSUM chunk-by-chunk, eagerly copied
            to SBUF fp32 with the causal mask folded into the last 256 cols.
            Returns the SBUF fp32 scores tile."""
            cols = (2 * jj + 2) * P
            sizes = _chunk_sizes(cols)
            s32 = s32p.tile([P, cols], f32, tag="s32", name=f"s32_{jj}")
            for c, sz in enumerate(sizes):
                ch = work.tile([P, 512], f32, tag="wk", name=f"sch{jj}_{c}")
                for o in range(OT):
                    nc.tensor.matmul(
                        ch[:, :sz],
                        lhsT=qT_sb[:, o, jj * P:(jj + 1) * P],
                        rhs=kT_sb[:, o, c * 512:c * 512 + sz],
                        start=(o == 0),
                        stop=(o == OT - 1),
                    )
                # copy to SBUF; the final 256 columns get the additive mask
                lo = c * 512
                hi = lo + sz
                mstart = cols - 256
                if hi <= mstart:
                    nc.vector.tensor_copy(out=s32[:, lo:hi], in_=ch[:, :sz])
                else:
                    plain = max(0, mstart - lo)
                    if plain:
                        nc.vector.tensor_copy(
                            out=s32[:, lo:lo + plain], in_=ch[:, :plain]
                        )
                    moff = lo + plain - mstart
                    nc.vector.tensor_tensor(
                        s32[:, lo + plain:hi],
                        ch[:, plain:sz],
                        mask_sb[:, moff:moff + (sz - plain)],
                        ADD,
                    )
            return s32

        # ---- software-pipelined attention slots ----
        emit_v_build(0)
        s32_cur = emit_s(0)

        for j in range(NSLOT):
            L = 2 * j + 2
            cols = L * P

            # softmax(j): row max (DVE) then exp with bias=-max (ACT)
            negm = scal.tile([P, 1], f32, tag="negm", name=f"negm{j}")
            nc.vector.reduce_max(negm[:], s32_cur[:, :cols], axis=AXX, negate=True)
            p_sb = ppool.tile([P, cols], f16, tag="p", name=f"p{j}")
            ell = scal.tile([P, 1], f32, tag="ell", name=f"ell{j}")
            nc.scalar.activation(
                p_sb[:], s32_cur[:, :cols], EXP,
                bias=negm[:], scale=1.0, accum_out=ell[:],
            )

            # next slot's V tiles and scores keep PE busy during softmax(j)
            if j + 1 < NSLOT:
                emit_v_build(j + 1)
                s32_next = emit_s(j + 1)
            else:
                s32_next = None

            # transpose P(j) 128-col tile at a time
            pts = []
            for kt in range(L):
                pt_ps = work.tile([P, P], f16, tag="wk", name=f"ptps{j}_{kt}")
                nc.tensor.transpose(
                    pt_ps[:], p_sb[:, kt * P:(kt + 1) * P], ident[:]
                )
                pt_sb = ptpool.tile([P, P], f16, tag="pt", name=f"pt{j}_{kt}")
                nc.vector.tensor_copy(out=pt_sb[:], in_=pt_ps[:])
                pts.append(pt_sb)

            rinv = scal.tile([P, 1], f32, tag="rinv", name=f"rinv{j}")
            nc.vector.reciprocal(rinv[:], ell[:])

            # O(j) = P^T.T @ V accumulation over key tiles
            o_ps = opsum.tile([P, D], f32, tag="o", name=f"ops{j}")
            for kt in range(L):
                for c in range(2):
                    nc.tensor.matmul(
                        o_ps[:, c * 512:(c + 1) * 512],
                        lhsT=pts[kt][:],
                        rhs=v_sb[:, kt, c * 512:(c + 1) * 512],
                        start=(kt == 0),
                        stop=(kt == L - 1),
                    )

            o_sb = big.tile([P, D], f32, tag="big", name=f"osb{j}")
            nc.scalar.mul(o_sb[:], o_ps[:], rinv[:])
            nc.sync.dma_start(out=out_d[j * P:(j + 1) * P, :], in_=o_sb[:])

            s32_cur = s32_next

    nc.compile()
    _PROG_CACHE["nc"] = nc
    return nc


def make_in_maps(x, Wq, Wk, Wv):
    """Host-side sharding: returns per-core input dicts (core c = 2*b + h)."""
    x = np.asarray(x, dtype=np.float32)
    wq16 = np.asarray(Wq, dtype=np.float32).astype(np.float16)
    wk16 = np.asarray(Wk, dtype=np.float32).astype(np.float16)
    wv16 = np.asarray(Wv, dtype=np.float32).astype(np.float16)

    tri = np.where(
        np.arange(P)[None, :] <= np.arange(P)[:, None], 0.0, MASK_NEG
    ).astype(np.float32)
    masks = []
    for h in range(2):
        m = np.zeros((P, 2 * P), dtype=np.float32)
        if h == 0:
            m[:, :P] = tri
            m[:, P:] = MASK_NEG
        else:
            m[:, :P] = 0.0
            m[:, P:] = tri
        masks.append(m)

    in_maps = []
    for b in range(NB):
        xt = np.ascontiguousarray(x[b].T)          # [D, SEQ] f32
        xt16 = xt.astype(np.float16)
        for h in range(2):
            qcols = np.concatenate(
                [np.arange((2 * j + h) * P, (2 * j + h + 1) * P)
                 for j in range(NSLOT)]
            )
            xq = np.ascontiguousarray((xt[:, qcols] * SCALE).astype(np.float16))
            in_maps.append({
                "xkv": xt16,
                "xq": xq,
                "wq": wq16,
                "wk": wk16,
                "wv": wv16,
                "mask": masks[h],
            })
    return in_maps


def assemble_output(results):
    """Gather per-core [NSLOT*P, D] outputs into the full [NB, SEQ, D]."""
    out = np.empty((NB, SEQ, D), dtype=np.float32)
    for b in range(NB):
        for h in range(2):
            r = results[2 * b + h]["out"]
            for j in range(NSLOT):
                g = 2 * j + h
                out[b, g * P:(g + 1) * P, :] = r[j * P:(j + 1) * P, :]
    return out


def run(inputs, trace=False, tmpdir=None):
    """Build, run on 8 cores, gather. Returns (output, BassKernelResults)."""
    _install_ntff_hook()
    from concourse.bass_utils import run_bass_kernel_spmd

    nc = build_program()
    in_maps = make_in_maps(
        inputs["x"], inputs["Wq"], inputs["Wk"], inputs["Wv"]
    )
    kw = {}
    if trace:
        kw["trace"] = True
        if tmpdir is not None:
            kw["tmpdir"] = tmpdir
    res = run_bass_kernel_spmd(nc, in_maps, list(range(8)), **kw)
    return assemble_output(res.results), res


def kernel(**inputs):
    out, _ = run(inputs, trace=False)
    return out


# revision 6
# speedup vs baseline: 1.2411x; 1.0802x over previous
"""Causal single-head attention (b=4, s=2048, d=1024) on 8 trn2 NeuronCores.

Sharding: data-parallel over batch (4) x 2-way split per batch.
Core c = 2*b + h handles batch b; within the pair:
  - K^T and V are computed for HALF the sequence each (contiguous halves,
    rank h covers seq columns [h*1024, (h+1)*1024)) and exchanged via a
    pairwise AllGather (replica groups [[0,1],[2,3],[4,5],[6,7]]).
  - Q^T is computed for the core's own 8 query tiles {2j+h : j=0..7}.
Causality is handled by rounding each slot's key range up to (2j+2)
tiles plus a per-core additive mask (data, not program) over the last
256 columns, keeping the SPMD program identical across cores.

All matmuls run in fp16 (1 cyc/row on PE, fp32 PSUM accumulation);
softmax runs in fp32 on ACT/DVE. Attention slots are software-pipelined:
while PE computes S(j+1), ACT/DVE run softmax(j); PE then transposes
P(j) and accumulates O(j) = P^T.T @ V without stalling.
"""

import sys
import types

import numpy as np

P = 128
SEQ = 2048
D = 1024
NB = 4
NSLOT = 8          # query tiles per core
IT = D // P        # 8 contraction tiles (d_in)
OT = D // P        # 8 output tiles (d_out)
ST = SEQ // P      # 16 seq tiles
HT = ST // 2       # 8 seq tiles per rank
HCOL = HT * P      # 1024 seq columns per rank
MASK_NEG = -30000.0
SCALE = 1.0 / 32.0  # 1/sqrt(d_out)

_PROG_CACHE = {}


def _install_ntff_hook():
    """Register the NTFF profile hook this image's antenv lacks (best effort)."""
    try:
        import antenv.axon_hooks  # noqa: F401
        return
    except ImportError:
        pass
    try:
        import trn_agent_boot.trn_boot as tb
        hook = tb._ntff_profile_via_ctypes('/opt/axon/libaxon_pjrt.so')
        mod = types.ModuleType('antenv.axon_hooks')
        mod._hook = hook
        mod.get_axon_ntff_profile_hook = lambda: mod._hook

        def _set(h):
            mod._hook = h
        mod.set_axon_ntff_profile_hook = _set
        sys.modules['antenv.axon_hooks'] = mod
    except Exception:
        pass


def _chunk_sizes(cols):
    sizes = []
    off = 0
    while off < cols:
        sizes.append(min(512, cols - off))
        off += sizes[-1]
    return sizes


def build_program():
    """Build + compile the single SPMD Bass program (cached)."""
    if "nc" in _PROG_CACHE:
        return _PROG_CACHE["nc"]

    from contextlib import ExitStack

    import concourse.mybir as mybir
    from concourse import bacc
    from concourse.masks import make_identity
    from concourse.tile import TileContext

    f32 = mybir.dt.float32
    f16 = mybir.dt.float16
    ADD = mybir.AluOpType.add
    AXX = mybir.AxisListType.X
    EXP = mybir.ActivationFunctionType.Exp
    GROUPS = [[0, 1], [2, 3], [4, 5], [6, 7]]

    nc = bacc.Bacc("TRN2", target_bir_lowering=False, debug=False, num_devices=8)

    xkv_d = nc.dram_tensor("xkv", [D, HCOL], f16, kind="ExternalInput").ap()
    xq_d = nc.dram_tensor("xq", [D, NSLOT * P], f16, kind="ExternalInput").ap()
    wq_d = nc.dram_tensor("wq", [D, D], f16, kind="ExternalInput").ap()
    wk_d = nc.dram_tensor("wk", [D, D], f16, kind="ExternalInput").ap()
    wv_d = nc.dram_tensor("wv", [D, D], f16, kind="ExternalInput").ap()
    mask_d = nc.dram_tensor("mask", [P, 2 * P], f32, kind="ExternalInput").ap()
    out_d = nc.dram_tensor("out", [NSLOT * P, D], f32, kind="ExternalOutput").ap()

    # DRAM staging for the pairwise K/V exchange
    kt_half_d = nc.dram_tensor("kt_half", [OT, P, HCOL], f16).ap()
    v_half_d = nc.dram_tensor("v_half", [HT, P, D], f16).ap()
    kt_all_d = nc.dram_tensor("kt_all", [2, OT, P, HCOL], f16).ap()
    v_all_d = nc.dram_tensor("v_all", [2, HT, P, D], f16).ap()

    with TileContext(nc) as tc, ExitStack() as ctx:
        const = ctx.enter_context(tc.tile_pool(name="const", bufs=1))
        persist = ctx.enter_context(tc.tile_pool(name="persist", bufs=1))
        big = ctx.enter_context(tc.tile_pool(name="big", bufs=4))
        s32p = ctx.enter_context(tc.tile_pool(name="s32p", bufs=2))
        ppool = ctx.enter_context(tc.tile_pool(name="ppool", bufs=2))
        ptpool = ctx.enter_context(tc.tile_pool(name="ptpool", bufs=18))
        scal = ctx.enter_context(tc.tile_pool(name="scal", bufs=16))
        work = ctx.enter_context(tc.tile_pool(name="work", bufs=6, space="PSUM"))
        opsum = ctx.enter_context(tc.tile_pool(name="opsum", bufs=1, space="PSUM"))

        mask_sb = const.tile([P, 2 * P], f32, tag="mask")
        nc.sync.dma_start(out=mask_sb[:], in_=mask_d)
        ident = const.tile([P, P], f16, tag="ident")
        make_identity(nc, ident[:])

        # ---- input DMAs, ordered so K-build can start ASAP ----
        wk_sb = big.tile([P, IT, D], f16, tag="big", name="wk_sb")
        wq_sb = big.tile([P, IT, D], f16, tag="big", name="wq_sb")
        wv_sb = big.tile([P, IT, D], f16, tag="big", name="wv_sb")
        xq_sb = big.tile([P, IT, NSLOT * P], f16, tag="big", name="xq_sb")
        xkv_sb = persist.tile([P, IT, HCOL], f16, tag="xkv")

        xkv_t = xkv_d.rearrange("(i p) s -> i p s", p=P)
        wk_t = wk_d.rearrange("(i p) o -> i p o", p=P)
        for i in range(IT):
            nc.sync.dma_start(out=wk_sb[:, i], in_=wk_t[i])
            nc.sync.dma_start(out=xkv_sb[:, i], in_=xkv_t[i])
        wv_t = wv_d.rearrange("(i p) o -> i p o", p=P)
        for i in range(IT):
            nc.sync.dma_start(out=wv_sb[:, i], in_=wv_t[i])
        wq_t = wq_d.rearrange("(i p) o -> i p o", p=P)
        xq_t = xq_d.rearrange("(i p) s -> i p s", p=P)
        for i in range(IT):
            nc.sync.dma_start(out=wq_sb[:, i], in_=wq_t[i])
            nc.sync.dma_start(out=xq_sb[:, i], in_=xq_t[i])

        kT_sb = persist.tile([P, OT, SEQ], f16, tag="kT")
        qT_sb = persist.tile([P, OT, NSLOT * P], f16, tag="qT")
        v_sb = persist.tile([P, ST, D], f16, tag="v")

        # ---- local K^T half: kT[o, s_loc] = sum_i wk[i, o] * x[s_loc, i] ----
        # written position-independently into kT_sb[:, o, 0:HCOL], staged out
        # to DRAM, exchanged, then both rank halves land in global positions.
        for o in range(OT):
            chunks = [work.tile([P, 512], f32, tag="wk", name=f"kch{o}_{c}")
                      for c in range(2)]
            for i in range(IT):
                lhsT = wk_sb[:, i, o * P:(o + 1) * P]
                for c in range(2):
                    nc.tensor.matmul(
                        chunks[c][:],
                        lhsT=lhsT,
                        rhs=xkv_sb[:, i, c * 512:(c + 1) * 512],
                        start=(i == 0),
                        stop=(i == IT - 1),
                    )
            for c in range(2):
                nc.vector.tensor_copy(
                    out=kT_sb[:, o, c * 512:(c + 1) * 512], in_=chunks[c][:]
                )
            nc.sync.dma_start(out=kt_half_d[o], in_=kT_sb[:, o, 0:HCOL])

        nc.gpsimd.collective_compute(
            "AllGather",
            mybir.AluOpType.bypass,
            replica_groups=GROUPS,
            ins=[kt_half_d[:]],
            outs=[kt_all_d[:]],
        )

        # ---- local V half: v[s_loc, o] = sum_i x[s_loc, i] wv[i, o] ----
        for st in range(HT):
            vch = [work.tile([P, 512], f32, tag="wk", name=f"vch{st}_{c}")
                   for c in range(2)]
            for i in range(IT):
                lhsT = xkv_sb[:, i, st * P:(st + 1) * P]
                for c in range(2):
                    nc.tensor.matmul(
                        vch[c][:],
                        lhsT=lhsT,
                        rhs=wv_sb[:, i, c * 512:(c + 1) * 512],
                        start=(i == 0),
                        stop=(i == IT - 1),
                    )
            for c in range(2):
                nc.scalar.copy(
                    out=v_sb[:, st, c * 512:(c + 1) * 512], in_=vch[c][:]
                )
            nc.sync.dma_start(out=v_half_d[st], in_=v_sb[:, st, :])

        nc.gpsimd.collective_compute(
            "AllGather",
            mybir.AluOpType.bypass,
            replica_groups=GROUPS,
            ins=[v_half_d[:]],
            outs=[v_all_d[:]],
        )

        # ---- Q^T build (xq pre-scaled by 1/32 on host); overlaps exchange ----
        for o in range(OT):
            chunks = [work.tile([P, 512], f32, tag="wk", name=f"qch{o}_{c}")
                      for c in range(2)]
            for i in range(IT):
                lhsT = wq_sb[:, i, o * P:(o + 1) * P]
                for c in range(2):
                    nc.tensor.matmul(
                        chunks[c][:],
                        lhsT=lhsT,
                        rhs=xq_sb[:, i, c * 512:(c + 1) * 512],
                        start=(i == 0),
                        stop=(i == IT - 1),
                    )
            for c in range(2):
                nc.vector.tensor_copy(
                    out=qT_sb[:, o, c * 512:(c + 1) * 512], in_=chunks[c][:]
                )

        # ---- land the exchanged K^T / V halves in global positions ----
        for r in range(2):
            for o in range(OT):
                nc.sync.dma_start(
                    out=kT_sb[:, o, r * HCOL:(r + 1) * HCOL],
                    in_=kt_all_d[r, o],
                )
        for r in range(2):
            for st in range(HT):
                nc.sync.dma_start(
                    out=v_sb[:, r * HT + st, :], in_=v_all_d[r, st]
                )

        def emit_s(jj):
            """S(jj) = Q^T.T @ K^T into PSUM chunk-by-chunk, eagerly copied
            to SBUF fp32 with the causal mask folded into the last 256 cols."""
            cols = (2 * jj + 2) * P
            sizes = _chunk_sizes(cols)
            s32 = s32p.tile([P, cols], f32, tag="s32", name=f"s32_{jj}")
            for c, sz in enumerate(sizes):
                ch = work.tile([P, 512], f32, tag="wk", name=f"sch{jj}_{c}")
                for o in range(OT):
                    nc.tensor.matmul(
                        ch[:, :sz],
                        lhsT=qT_sb[:, o, jj * P:(jj + 1) * P],
                        rhs=kT_sb[:, o, c * 512:c * 512 + sz],
                        start=(o == 0),
                        stop=(o == OT - 1),
                    )
                lo = c * 512
                hi = lo + sz
                mstart = cols - 256
                if hi <= mstart:
                    nc.vector.tensor_copy(out=s32[:, lo:hi], in_=ch[:, :sz])
                else:
                    plain = max(0, mstart - lo)
                    if plain:
                        nc.vector.tensor_copy(
                            out=s32[:, lo:lo + plain], in_=ch[:, :plain]
                        )
                    moff = lo + plain - mstart
                    nc.vector.tensor_tensor(
                        s32[:, lo + plain:hi],
                        ch[:, plain:sz],
                        mask_sb[:, moff:moff + (sz - plain)],
                        ADD,
                    )
            return s32

        # ---- software-pipelined attention slots ----
        s32_cur = emit_s(0)

        for j in range(NSLOT):
            L = 2 * j + 2
            cols = L * P

            # softmax(j): row max (DVE) then exp with bias=-max (ACT)
            negm = scal.tile([P, 1], f32, tag="negm", name=f"negm{j}")
            nc.vector.reduce_max(negm[:], s32_cur[:, :cols], axis=AXX, negate=True)
            p_sb = ppool.tile([P, cols], f16, tag="p", name=f"p{j}")
            ell = scal.tile([P, 1], f32, tag="ell", name=f"ell{j}")
            nc.scalar.activation(
                p_sb[:], s32_cur[:, :cols], EXP,
                bias=negm[:], scale=1.0, accum_out=ell[:],
            )

            # next slot's scores keep PE busy during softmax(j)
            s32_next = emit_s(j + 1) if j + 1 < NSLOT else None

            # transpose P(j) 128-col tile at a time
            pts = []
            for kt in range(L):
                pt_ps = work.tile([P, P], f16, tag="wk", name=f"ptps{j}_{kt}")
                nc.tensor.transpose(
                    pt_ps[:], p_sb[:, kt * P:(kt + 1) * P], ident[:]
                )
                pt_sb = ptpool.tile([P, P], f16, tag="pt", name=f"pt{j}_{kt}")
                nc.vector.tensor_copy(out=pt_sb[:], in_=pt_ps[:])
                pts.append(pt_sb)

            rinv = scal.tile([P, 1], f32, tag="rinv", name=f"rinv{j}")
            nc.vector.reciprocal(rinv[:], ell[:])

            # O(j) = P^T.T @ V accumulation over key tiles
            o_ps = opsum.tile([P, D], f32, tag="o", name=f"ops{j}")
            for kt in range(L):
                for c in range(2):
                    nc.tensor.matmul(
                        o_ps[:, c * 512:(c + 1) * 512],
                        lhsT=pts[kt][:],
                        rhs=v_sb[:, kt, c * 512:(c + 1) * 512],
                        start=(kt == 0),
                        stop=(kt == L - 1),
                    )

            o_sb = big.tile([P, D], f32, tag="big", name=f"osb{j}")
            nc.scalar.mul(o_sb[:], o_ps[:], rinv[:])
            nc.sync.dma_start(out=out_d[j * P:(j + 1) * P, :], in_=o_sb[:])

            s32_cur = s32_next

    nc.compile()
    _PROG_CACHE["nc"] = nc
    return nc


def make_in_maps(x, Wq, Wk, Wv):
    """Host-side sharding: returns per-core input dicts (core c = 2*b + h)."""
    x = np.asarray(x, dtype=np.float32)
    wq16 = np.asarray(Wq, dtype=np.float32).astype(np.float16)
    wk16 = np.asarray(Wk, dtype=np.float32).astype(np.float16)
    wv16 = np.asarray(Wv, dtype=np.float32).astype(np.float16)

    tri = np.where(
        np.arange(P)[None, :] <= np.arange(P)[:, None], 0.0, MASK_NEG
    ).astype(np.float32)
    masks = []
    for h in range(2):
        m = np.zeros((P, 2 * P), dtype=np.float32)
        if h == 0:
            m[:, :P] = tri
            m[:, P:] = MASK_NEG
        else:
            m[:, :P] = 0.0
            m[:, P:] = tri
        masks.append(m)

    in_maps = []
    for b in range(NB):
        xt = np.ascontiguousarray(x[b].T)          # [D, SEQ] f32
        xt16 = xt.astype(np.float16)
        for h in range(2):
            qcols = np.concatenate(
                [np.arange((2 * j + h) * P, (2 * j + h + 1) * P)
                 for j in range(NSLOT)]
            )
            xq = np.ascontiguousarray((xt[:, qcols] * SCALE).astype(np.float16))
            in_maps.append({
                "xkv": np.ascontiguousarray(xt16[:, h * HCOL:(h + 1) * HCOL]),
                "xq": xq,
                "wq": wq16,
                "wk": wk16,
                "wv": wv16,
                "mask": masks[h],
            })
    return in_maps


def assemble_output(results):
    """Gather per-core [NSLOT*P, D] outputs into the full [NB, SEQ, D]."""
    out = np.empty((NB, SEQ, D), dtype=np.float32)
    for b in range(NB):
        for h in range(2):
            r = results[2 * b + h]["out"]
            for j in range(NSLOT):
                g = 2 * j + h
                out[b, g * P:(g + 1) * P, :] = r[j * P:(j + 1) * P, :]
    return out


def run(inputs, trace=False, tmpdir=None):
    """Build, run on 8 cores, gather. Returns (output, BassKernelResults)."""
    _install_ntff_hook()
    from concourse.bass_utils import run_bass_kernel_spmd

    nc = build_program()
    in_maps = make_in_maps(
        inputs["x"], inputs["Wq"], inputs["Wk"], inputs["Wv"]
    )
    kw = {}
    if trace:
        kw["trace"] = True
        if tmpdir is not None:
            kw["tmpdir"] = tmpdir
    res = run_bass_kernel_spmd(nc, in_maps, list(range(8)), **kw)
    return assemble_output(res.results), res


def kernel(**inputs):
    out, _ = run(inputs, trace=False)
    return out


# revision 8
# speedup vs baseline: 1.3008x; 1.0481x over previous
"""Causal single-head attention (b=4, s=2048, d=1024) on 8 trn2 NeuronCores.

Sharding: data-parallel over batch (4) x 2-way key split per batch.
Core c = 2*b + h handles batch b and KEY tiles {2m+h : m=0..7} (128-row
tiles, interleaved so causal work stays balanced). Each core:
  - computes K^T and V only for its own 8 key tiles (no duplication,
    no cross-core exchange),
  - computes Q^T for ALL 16 query tiles (Q projection is duplicated
    across the pair - it is half the cost of K+V),
  - runs a partial causal softmax over its key half for every query
    tile, emitting the normalized partial output O_h plus the row
    statistics (max m, sum l).
The host then merges the two partials per batch with a numerically
exact log-sum-exp combine.

Causality per query tile t over local key tiles 0..t//2: the last local
tile is either the diagonal (triangular mask), fully visible, or fully
masked, depending only on parity(t) and the core's rank - handled by a
per-core additive mask tensor (data, not program), so the SPMD program
is identical across all 8 cores.

All matmuls run in fp16 (1 cyc/row on PE, fp32 PSUM accumulation);
softmax runs in fp32 on ACT/DVE. Slots are software-pipelined: while PE
computes S(t+1), ACT/DVE run softmax(t); PE then transposes P(t) and
accumulates O(t) = P^T.T @ V without stalling.
"""

import sys
import types

import numpy as np

P = 128
SEQ = 2048
D = 1024
NB = 4
QT = SEQ // P      # 16 query tiles per core (all of them)
IT = D // P        # 8 contraction tiles (d_in)
OT = D // P        # 8 output tiles (d_out)
HT = QT // 2       # 8 key tiles per core
HCOL = HT * P      # 1024 local key columns
MASK_NEG = -30000.0
SCALE = 1.0 / 32.0  # 1/sqrt(d_out)

_PROG_CACHE = {}


def _install_ntff_hook():
    """Register the NTFF profile hook this image's antenv lacks (best effort)."""
    try:
        import antenv.axon_hooks  # noqa: F401
        return
    except ImportError:
        pass
    try:
        import trn_agent_boot.trn_boot as tb
        hook = tb._ntff_profile_via_ctypes('/opt/axon/libaxon_pjrt.so')
        mod = types.ModuleType('antenv.axon_hooks')
        mod._hook = hook
        mod.get_axon_ntff_profile_hook = lambda: mod._hook

        def _set(h):
            mod._hook = h
        mod.set_axon_ntff_profile_hook = _set
        sys.modules['antenv.axon_hooks'] = mod
    except Exception:
        pass


def build_program():
    """Build + compile the single SPMD Bass program (cached)."""
    if "nc" in _PROG_CACHE:
        return _PROG_CACHE["nc"]

    from contextlib import ExitStack

    import concourse.mybir as mybir
    from concourse import bacc
    from concourse.masks import make_identity
    from concourse.tile import TileContext

    f32 = mybir.dt.float32
    f16 = mybir.dt.float16
    ADD = mybir.AluOpType.add
    AXX = mybir.AxisListType.X
    EXP = mybir.ActivationFunctionType.Exp

    nc = bacc.Bacc("TRN2", target_bir_lowering=False, debug=False, num_devices=8)

    # xk: the core's interleaved key-half columns of X^T (compacted);
    # xq: full X^T pre-scaled by 1/32.
    xk_d = nc.dram_tensor("xk", [D, HCOL], f16, kind="ExternalInput").ap()
    xq_d = nc.dram_tensor("xq", [D, SEQ], f16, kind="ExternalInput").ap()
    wq_d = nc.dram_tensor("wq", [D, D], f16, kind="ExternalInput").ap()
    wk_d = nc.dram_tensor("wk", [D, D], f16, kind="ExternalInput").ap()
    wv_d = nc.dram_tensor("wv", [D, D], f16, kind="ExternalInput").ap()
    mask_d = nc.dram_tensor("mask", [2, P, P], f32, kind="ExternalInput").ap()
    out_d = nc.dram_tensor("out", [SEQ, D], f32, kind="ExternalOutput").ap()
    ml_d = nc.dram_tensor("ml", [P, 2 * QT], f32, kind="ExternalOutput").ap()

    with TileContext(nc) as tc, ExitStack() as ctx:
        const = ctx.enter_context(tc.tile_pool(name="const", bufs=1))
        persist = ctx.enter_context(tc.tile_pool(name="persist", bufs=1))
        wpool = ctx.enter_context(tc.tile_pool(name="wpool", bufs=3))
        s32p = ctx.enter_context(tc.tile_pool(name="s32p", bufs=2))
        ppool = ctx.enter_context(tc.tile_pool(name="ppool", bufs=2))
        ptpool = ctx.enter_context(tc.tile_pool(name="ptpool", bufs=18))
        scal = ctx.enter_context(tc.tile_pool(name="scal", bufs=24))
        work = ctx.enter_context(tc.tile_pool(name="work", bufs=6, space="PSUM"))
        opsum = ctx.enter_context(tc.tile_pool(name="opsum", bufs=1, space="PSUM"))

        mask_sb = const.tile([P, 2 * P], f32, tag="mask")
        nc.sync.dma_start(out=mask_sb[:, 0:P], in_=mask_d[0])
        nc.sync.dma_start(out=mask_sb[:, P:2 * P], in_=mask_d[1])
        ident = const.tile([P, P], f16, tag="ident")
        make_identity(nc, ident[:])

        # ---- input DMAs, ordered so K-build can start ASAP ----
        wk_sb = wpool.tile([P, IT, D], f16, tag="w", name="wk_sb")
        wv_sb = wpool.tile([P, IT, D], f16, tag="w", name="wv_sb")
        wq_sb = wpool.tile([P, IT, D], f16, tag="w", name="wq_sb")
        xk_sb = persist.tile([P, IT, HCOL], f16, tag="xk")
        xq_sb = persist.tile([P, IT, SEQ], f16, tag="xq")

        xk_t = xk_d.rearrange("(i p) s -> i p s", p=P)
        wk_t = wk_d.rearrange("(i p) o -> i p o", p=P)
        for i in range(IT):
            nc.sync.dma_start(out=wk_sb[:, i], in_=wk_t[i])
            nc.sync.dma_start(out=xk_sb[:, i], in_=xk_t[i])
        wv_t = wv_d.rearrange("(i p) o -> i p o", p=P)
        for i in range(IT):
            nc.sync.dma_start(out=wv_sb[:, i], in_=wv_t[i])
        wq_t = wq_d.rearrange("(i p) o -> i p o", p=P)
        xq_t = xq_d.rearrange("(i p) s -> i p s", p=P)
        for i in range(IT):
            nc.sync.dma_start(out=wq_sb[:, i], in_=wq_t[i])
            nc.sync.dma_start(out=xq_sb[:, i], in_=xq_t[i])

        stats_sb = persist.tile([P, 2 * QT], f32, tag="stats")
        kT_sb = persist.tile([P, OT, HCOL], f16, tag="kT")
        qT_sb = persist.tile([P, OT, SEQ], f16, tag="qT")
        v_sb = persist.tile([P, HT, D], f16, tag="v")

        # ---- local K^T: kT[o, m] = sum_i wk[i, o] * xk[i, m] ----
        for o in range(OT):
            chunks = [work.tile([P, 512], f32, tag="wk", name=f"kch{o}_{c}")
                      for c in range(2)]
            for i in range(IT):
                lhsT = wk_sb[:, i, o * P:(o + 1) * P]
                for c in range(2):
                    nc.tensor.matmul(
                        chunks[c][:],
                        lhsT=lhsT,
                        rhs=xk_sb[:, i, c * 512:(c + 1) * 512],
                        start=(i == 0),
                        stop=(i == IT - 1),
                    )
            for c in range(2):
                nc.vector.tensor_copy(
                    out=kT_sb[:, o, c * 512:(c + 1) * 512], in_=chunks[c][:]
                )

        # ---- local V: v[m, o] = sum_i xk[i, m] * wv[i, o] ----
        for st in range(HT):
            vch = [work.tile([P, 512], f32, tag="wk", name=f"vch{st}_{c}")
                   for c in range(2)]
            for i in range(IT):
                lhsT = xk_sb[:, i, st * P:(st + 1) * P]
                for c in range(2):
                    nc.tensor.matmul(
                        vch[c][:],
                        lhsT=lhsT,
                        rhs=wv_sb[:, i, c * 512:(c + 1) * 512],
                        start=(i == 0),
                        stop=(i == IT - 1),
                    )
            for c in range(2):
                nc.scalar.copy(
                    out=v_sb[:, st, c * 512:(c + 1) * 512], in_=vch[c][:]
                )

        # ---- full Q^T (xq pre-scaled by 1/32 on host) ----
        for o in range(OT):
            chunks = [work.tile([P, 512], f32, tag="wk", name=f"qch{o}_{c}")
                      for c in range(4)]
            for i in range(IT):
                lhsT = wq_sb[:, i, o * P:(o + 1) * P]
                for c in range(4):
                    nc.tensor.matmul(
                        chunks[c][:],
                        lhsT=lhsT,
                        rhs=xq_sb[:, i, c * 512:(c + 1) * 512],
                        start=(i == 0),
                        stop=(i == IT - 1),
                    )
            for c in range(4):
                nc.vector.tensor_copy(
                    out=qT_sb[:, o, c * 512:(c + 1) * 512], in_=chunks[c][:]
                )

        def emit_s(t):
            """Partial scores S(t) over local key tiles 0..t//2, eagerly
            copied to SBUF fp32 with the parity mask on the last 128 cols."""
            cols = (t // 2 + 1) * P
            s32 = s32p.tile([P, 1024], f32, tag="s32", name=f"s32_{t}")
            off = 0
            while off < cols:
                sz = min(512, cols - off)
                ch = work.tile([P, 512], f32, tag="wk", name=f"sch{t}_{off}")
                for o in range(OT):
                    nc.tensor.matmul(
                        ch[:, :sz],
                        lhsT=qT_sb[:, o, t * P:(t + 1) * P],
                        rhs=kT_sb[:, o, off:off + sz],
                        start=(o == 0),
                        stop=(o == OT - 1),
                    )
                # copy to SBUF; the final 128 columns get the parity mask
                mstart = cols - P
                lo, hi = off, off + sz
                plain = min(hi, mstart) - lo
                if plain > 0:
                    nc.vector.tensor_copy(
                        out=s32[:, lo:lo + plain], in_=ch[:, :plain]
                    )
                if hi > mstart:
                    moff = max(0, mstart - lo)
                    par = t % 2
                    nc.vector.tensor_tensor(
                        s32[:, max(lo, mstart):hi],
                        ch[:, moff:sz],
                        mask_sb[:, par * P:par * P + (hi - max(lo, mstart))],
                        ADD,
                    )
                off += sz
            return s32

        # ---- software-pipelined attention over all 16 query tiles ----
        s32_cur = emit_s(0)

        for t in range(QT):
            L = t // 2 + 1     # local key tiles
            cols = L * P

            # softmax(t): row max (DVE) then exp with bias=-max (ACT);
            # -max and sum land in stats_sb (DMA'd out once at the end)
            negm = stats_sb[:, 2 * t:2 * t + 1]
            nc.vector.reduce_max(negm, s32_cur[:, :cols], axis=AXX, negate=True)
            p_sb = ppool.tile([P, 1024], f16, tag="p", name=f"p{t}")
            ell = stats_sb[:, 2 * t + 1:2 * t + 2]
            nc.scalar.activation(
                p_sb[:, :cols], s32_cur[:, :cols], EXP,
                bias=negm, scale=1.0, accum_out=ell,
            )

            # next slot's scores keep PE busy during softmax(t)
            s32_next = emit_s(t + 1) if t + 1 < QT else None

            # transpose P(t) 128-col tile at a time
            pts = []
            for kt in range(L):
                pt_ps = work.tile([P, P], f16, tag="wk", name=f"ptps{t}_{kt}")
                nc.tensor.transpose(
                    pt_ps[:], p_sb[:, kt * P:(kt + 1) * P], ident[:]
                )
                pt_sb = ptpool.tile([P, P], f16, tag="pt", name=f"pt{t}_{kt}")
                nc.vector.tensor_copy(out=pt_sb[:], in_=pt_ps[:])
                pts.append(pt_sb)

            rinv = scal.tile([P, 1], f32, tag="rinv", name=f"rinv{t}")
            nc.vector.reciprocal(rinv[:], ell)

            # O(t) = P^T.T @ V accumulation over local key tiles
            o_ps = opsum.tile([P, D], f32, tag="o", name=f"ops{t}")
            for kt in range(L):
                for c in range(2):
                    nc.tensor.matmul(
                        o_ps[:, c * 512:(c + 1) * 512],
                        lhsT=pts[kt][:],
                        rhs=v_sb[:, kt, c * 512:(c + 1) * 512],
                        start=(kt == 0),
                        stop=(kt == L - 1),
                    )

            o_sb = wpool.tile([P, D], f32, tag="w", name=f"osb{t}")
            nc.scalar.mul(o_sb[:], o_ps[:], rinv[:])
            nc.sync.dma_start(out=out_d[t * P:(t + 1) * P, :], in_=o_sb[:])

            s32_cur = s32_next

        nc.sync.dma_start(out=ml_d[:], in_=stats_sb[:])

    nc.compile()
    _PROG_CACHE["nc"] = nc
    return nc


def make_in_maps(x, Wq, Wk, Wv):
    """Host-side sharding: returns per-core input dicts (core c = 2*b + h)."""
    x = np.asarray(x, dtype=np.float32)
    wq16 = np.asarray(Wq, dtype=np.float32).astype(np.float16)
    wk16 = np.asarray(Wk, dtype=np.float32).astype(np.float16)
    wv16 = np.asarray(Wv, dtype=np.float32).astype(np.float16)

    tri = np.where(
        np.arange(P)[None, :] <= np.arange(P)[:, None], 0.0, MASK_NEG
    ).astype(np.float32)
    full = np.full((P, P), MASK_NEG, dtype=np.float32)
    zero = np.zeros((P, P), dtype=np.float32)
    # mask[parity]: additive mask for the last local key tile of query
    # tile t (parity = t%2). Local tile u = 2*(t//2) + h:
    #   h=0: t even -> u==t (diagonal tri); t odd -> u==t-1 (visible)
    #   h=1: t even -> u==t+1 (fully masked); t odd -> u==t (diagonal tri)
    masks = [
        np.stack([tri, zero]),   # h = 0
        np.stack([full, tri]),   # h = 1
    ]

    in_maps = []
    for b in range(NB):
        xt = np.ascontiguousarray(x[b].T)          # [D, SEQ] f32
        xt16 = xt.astype(np.float16)
        xq = np.ascontiguousarray((xt * SCALE).astype(np.float16))
        for h in range(2):
            kcols = np.concatenate(
                [np.arange((2 * m + h) * P, (2 * m + h + 1) * P)
                 for m in range(HT)]
            )
            in_maps.append({
                "xk": np.ascontiguousarray(xt16[:, kcols]),
                "xq": xq,
                "wq": wq16,
                "wk": wk16,
                "wv": wv16,
                "mask": masks[h],
            })
    return in_maps


def assemble_output(results):
    """Log-sum-exp combine of the two partial softmax halves per batch."""
    out = np.empty((NB, SEQ, D), dtype=np.float32)
    for b in range(NB):
        r0 = results[2 * b]
        r1 = results[2 * b + 1]
        o0 = r0["out"].astype(np.float64)
        o1 = r1["out"].astype(np.float64)
        # ml is [P, 2*QT]: col 2t = -max, col 2t+1 = sum; q = t*128 + p
        ml0 = r0["ml"].astype(np.float64)
        ml1 = r1["ml"].astype(np.float64)
        m0 = -(ml0[:, 0::2].T.reshape(SEQ))
        l0 = ml0[:, 1::2].T.reshape(SEQ)
        m1 = -(ml1[:, 0::2].T.reshape(SEQ))
        l1 = ml1[:, 1::2].T.reshape(SEQ)
        mm = np.maximum(m0, m1)
        w0 = l0 * np.exp(m0 - mm)
        w1 = l1 * np.exp(m1 - mm)
        tot = w0 + w1
        w0 /= tot
        w1 /= tot
        out[b] = (o0 * w0[:, None] + o1 * w1[:, None]).astype(np.float32)
    return out


def run(inputs, trace=False, tmpdir=None):
    """Build, run on 8 cores, gather. Returns (output, BassKernelResults)."""
    _install_ntff_hook()
    from concourse.bass_utils import run_bass_kernel_spmd

    nc = build_program()
    in_maps = make_in_maps(
        inputs["x"], inputs["Wq"], inputs["Wk"], inputs["Wv"]
    )
    kw = {}
    if trace:
        kw["trace"] = True
        if tmpdir is not None:
            kw["tmpdir"] = tmpdir
    res = run_bass_kernel_spmd(nc, in_maps, list(range(8)), **kw)
    return assemble_output(res.results), res


def kernel(**inputs):
    out, _ = run(inputs, trace=False)
    return out


# revision 10
# speedup vs baseline: 1.3129x; 1.0093x over previous
"""Causal single-head attention (b=4, s=2048, d=1024) on 8 trn2 NeuronCores.

Sharding: data-parallel over batch (4) x 2-way key split per batch.
Core c = 2*b + h handles batch b and KEY tiles {2m+h : m=0..7} (128-row
tiles, interleaved so causal work stays balanced). Each core:
  - computes K^T and V only for its own 8 key tiles (no duplication,
    no cross-core exchange),
  - computes Q^T for ALL 16 query tiles (Q projection is duplicated
    across the pair - it is half the cost of K+V),
  - runs a partial causal softmax over its key half for every query
    tile, emitting the normalized partial output O_h plus the row
    statistics (max m, sum l).
The host then merges the two partials per batch with a numerically
exact log-sum-exp combine.

Causality per query tile t over local key tiles 0..t//2: the last local
tile is either the diagonal (triangular mask), fully visible, or fully
masked, depending only on parity(t) and the core's rank - handled by a
per-core additive mask tensor (data, not program), so the SPMD program
is identical across all 8 cores.

All matmuls run in fp16 (1 cyc/row on PE, fp32 PSUM accumulation);
softmax runs in fp32 on ACT/DVE. Slots are software-pipelined: while PE
computes S(t+1), ACT/DVE run softmax(t); PE then transposes P(t) and
accumulates O(t) = P^T.T @ V without stalling.
"""

import sys
import types

import numpy as np

P = 128
SEQ = 2048
D = 1024
NB = 4
QT = SEQ // P      # 16 query tiles per core (all of them)
IT = D // P        # 8 contraction tiles (d_in)
OT = D // P        # 8 output tiles (d_out)
HT = QT // 2       # 8 key tiles per core
HCOL = HT * P      # 1024 local key columns
MASK_NEG = -30000.0
SCALE = 1.0 / 32.0  # 1/sqrt(d_out)

_PROG_CACHE = {}


def _install_ntff_hook():
    """Register the NTFF profile hook this image's antenv lacks (best effort)."""
    try:
        import antenv.axon_hooks  # noqa: F401
        return
    except ImportError:
        pass
    try:
        import trn_agent_boot.trn_boot as tb
        hook = tb._ntff_profile_via_ctypes('/opt/axon/libaxon_pjrt.so')
        mod = types.ModuleType('antenv.axon_hooks')
        mod._hook = hook
        mod.get_axon_ntff_profile_hook = lambda: mod._hook

        def _set(h):
            mod._hook = h
        mod.set_axon_ntff_profile_hook = _set
        sys.modules['antenv.axon_hooks'] = mod
    except Exception:
        pass


def build_program():
    """Build + compile the single SPMD Bass program (cached)."""
    if "nc" in _PROG_CACHE:
        return _PROG_CACHE["nc"]

    from contextlib import ExitStack

    import concourse.mybir as mybir
    from concourse import bacc
    from concourse.masks import make_identity
    from concourse.tile import TileContext

    f32 = mybir.dt.float32
    f16 = mybir.dt.float16
    ADD = mybir.AluOpType.add
    AXX = mybir.AxisListType.X
    EXP = mybir.ActivationFunctionType.Exp

    nc = bacc.Bacc("TRN2", target_bir_lowering=False, debug=False, num_devices=8)

    # xk: the core's interleaved key-half columns of X^T (compacted);
    # xq: full X^T pre-scaled by 1/32.
    xk_d = nc.dram_tensor("xk", [D, HCOL], f16, kind="ExternalInput").ap()
    xq_d = nc.dram_tensor("xq", [D, SEQ], f16, kind="ExternalInput").ap()
    wq_d = nc.dram_tensor("wq", [D, D], f16, kind="ExternalInput").ap()
    wk_d = nc.dram_tensor("wk", [D, D], f16, kind="ExternalInput").ap()
    wv_d = nc.dram_tensor("wv", [D, D], f16, kind="ExternalInput").ap()
    mask_d = nc.dram_tensor("mask", [2, P, P], f32, kind="ExternalInput").ap()
    out_d = nc.dram_tensor("out", [SEQ, D], f32, kind="ExternalOutput").ap()
    ml_d = nc.dram_tensor("ml", [P, 2 * QT], f32, kind="ExternalOutput").ap()

    with TileContext(nc) as tc, ExitStack() as ctx:
        const = ctx.enter_context(tc.tile_pool(name="const", bufs=1))
        persist = ctx.enter_context(tc.tile_pool(name="persist", bufs=1))
        wpool = ctx.enter_context(tc.tile_pool(name="wpool", bufs=3))
        s32p = ctx.enter_context(tc.tile_pool(name="s32p", bufs=3))
        ppool = ctx.enter_context(tc.tile_pool(name="ppool", bufs=2))
        ptpool = ctx.enter_context(tc.tile_pool(name="ptpool", bufs=18))
        scal = ctx.enter_context(tc.tile_pool(name="scal", bufs=24))
        work = ctx.enter_context(tc.tile_pool(name="work", bufs=6, space="PSUM"))
        opsum = ctx.enter_context(tc.tile_pool(name="opsum", bufs=1, space="PSUM"))

        warm_sb = const.tile([P, 512], f16, tag="warm")
        nc.gpsimd.memset(warm_sb[:], 0.0)
        warm_ps = work.tile([P, 512], f32, tag="wk", name="warm_ps")
        for w in range(20):
            nc.tensor.matmul(
                warm_ps[:], lhsT=warm_sb[:, 0:P], rhs=warm_sb[:],
                start=(w == 0), stop=(w == 19),
            )

        mask_sb = const.tile([P, 2 * P], f32, tag="mask")
        nc.sync.dma_start(out=mask_sb[:, 0:P], in_=mask_d[0])
        nc.sync.dma_start(out=mask_sb[:, P:2 * P], in_=mask_d[1])
        ident = const.tile([P, P], f16, tag="ident")
        make_identity(nc, ident[:])

        # ---- input DMAs, ordered so K-build can start ASAP ----
        wk_sb = wpool.tile([P, IT, D], f16, tag="w", name="wk_sb")
        wv_sb = wpool.tile([P, IT, D], f16, tag="w", name="wv_sb")
        wq_sb = wpool.tile([P, IT, D], f16, tag="w", name="wq_sb")
        xk_sb = persist.tile([P, IT, HCOL], f16, tag="xk")
        xq_sb = persist.tile([P, IT, SEQ], f16, tag="xq")

        xk_t = xk_d.rearrange("(i p) s -> i p s", p=P)
        wk_t = wk_d.rearrange("(i p) o -> i p o", p=P)
        for i in range(IT):
            nc.sync.dma_start(out=wk_sb[:, i], in_=wk_t[i])
            nc.sync.dma_start(out=xk_sb[:, i], in_=xk_t[i])
        wv_t = wv_d.rearrange("(i p) o -> i p o", p=P)
        for i in range(IT):
            nc.sync.dma_start(out=wv_sb[:, i], in_=wv_t[i])
        wq_t = wq_d.rearrange("(i p) o -> i p o", p=P)
        xq_t = xq_d.rearrange("(i p) s -> i p s", p=P)
        for i in range(IT):
            nc.sync.dma_start(out=wq_sb[:, i], in_=wq_t[i])
            nc.sync.dma_start(out=xq_sb[:, i], in_=xq_t[i])

        stats_sb = persist.tile([P, 2 * QT], f32, tag="stats")
        kT_sb = persist.tile([P, OT, HCOL], f16, tag="kT")
        qT_sb = persist.tile([P, OT, SEQ], f16, tag="qT")
        v_sb = persist.tile([P, HT, D], f16, tag="v")

        # ---- local K^T: kT[o, m] = sum_i wk[i, o] * xk[i, m] ----
        for o in range(OT):
            chunks = [work.tile([P, 512], f32, tag="wk", name=f"kch{o}_{c}")
                      for c in range(2)]
            for i in range(IT):
                lhsT = wk_sb[:, i, o * P:(o + 1) * P]
                for c in range(2):
                    nc.tensor.matmul(
                        chunks[c][:],
                        lhsT=lhsT,
                        rhs=xk_sb[:, i, c * 512:(c + 1) * 512],
                        start=(i == 0),
                        stop=(i == IT - 1),
                    )
            for c in range(2):
                nc.vector.tensor_copy(
                    out=kT_sb[:, o, c * 512:(c + 1) * 512], in_=chunks[c][:]
                )

        # ---- local V: v[m, o] = sum_i xk[i, m] * wv[i, o] ----
        for st in range(HT):
            vch = [work.tile([P, 512], f32, tag="wk", name=f"vch{st}_{c}")
                   for c in range(2)]
            for i in range(IT):
                lhsT = xk_sb[:, i, st * P:(st + 1) * P]
                for c in range(2):
                    nc.tensor.matmul(
                        vch[c][:],
                        lhsT=lhsT,
                        rhs=wv_sb[:, i, c * 512:(c + 1) * 512],
                        start=(i == 0),
                        stop=(i == IT - 1),
                    )
            for c in range(2):
                nc.scalar.copy(
                    out=v_sb[:, st, c * 512:(c + 1) * 512], in_=vch[c][:]
                )

        # ---- full Q^T (xq pre-scaled by 1/32 on host) ----
        for o in range(OT):
            chunks = [work.tile([P, 512], f32, tag="wk", name=f"qch{o}_{c}")
                      for c in range(4)]
            for i in range(IT):
                lhsT = wq_sb[:, i, o * P:(o + 1) * P]
                for c in range(4):
                    nc.tensor.matmul(
                        chunks[c][:],
                        lhsT=lhsT,
                        rhs=xq_sb[:, i, c * 512:(c + 1) * 512],
                        start=(i == 0),
                        stop=(i == IT - 1),
                    )
            for c in range(4):
                nc.vector.tensor_copy(
                    out=qT_sb[:, o, c * 512:(c + 1) * 512], in_=chunks[c][:]
                )

        def emit_s(t):
            """Partial scores S(t) over local key tiles 0..t//2, eagerly
            copied to SBUF fp32 with the parity mask on the last 128 cols."""
            cols = (t // 2 + 1) * P
            s32 = s32p.tile([P, 1024], f32, tag="s32", name=f"s32_{t}")
            off = 0
            while off < cols:
                sz = min(512, cols - off)
                ch = work.tile([P, 512], f32, tag="wk", name=f"sch{t}_{off}")
                for o in range(OT):
                    nc.tensor.matmul(
                        ch[:, :sz],
                        lhsT=qT_sb[:, o, t * P:(t + 1) * P],
                        rhs=kT_sb[:, o, off:off + sz],
                        start=(o == 0),
                        stop=(o == OT - 1),
                    )
                # copy to SBUF; the final 128 columns get the parity mask
                mstart = cols - P
                lo, hi = off, off + sz
                plain = min(hi, mstart) - lo
                if plain > 0:
                    nc.vector.tensor_copy(
                        out=s32[:, lo:lo + plain], in_=ch[:, :plain]
                    )
                if hi > mstart:
                    moff = max(0, mstart - lo)
                    par = t % 2
                    nc.vector.tensor_tensor(
                        s32[:, max(lo, mstart):hi],
                        ch[:, moff:sz],
                        mask_sb[:, par * P:par * P + (hi - max(lo, mstart))],
                        ADD,
                    )
                off += sz
            return s32

        # ---- software-pipelined attention over all 16 query tiles ----
        s32q = [emit_s(0), emit_s(1)]

        for t in range(QT):
            s32_cur = s32q.pop(0)
            L = t // 2 + 1     # local key tiles
            cols = L * P

            # softmax(t): row max (DVE) then exp with bias=-max (ACT);
            # -max and sum land in stats_sb (DMA'd out once at the end)
            negm = stats_sb[:, 2 * t:2 * t + 1]
            nc.vector.reduce_max(negm, s32_cur[:, :cols], axis=AXX, negate=True)
            p_sb = ppool.tile([P, 1024], f16, tag="p", name=f"p{t}")
            ell = stats_sb[:, 2 * t + 1:2 * t + 2]
            nc.scalar.activation(
                p_sb[:, :cols], s32_cur[:, :cols], EXP,
                bias=negm, scale=1.0, accum_out=ell,
            )

            # scores two slots ahead keep PE busy during softmax(t)
            if t + 2 < QT:
                s32q.append(emit_s(t + 2))

            # transpose P(t) 128-col tile at a time
            pts = []
            for kt in range(L):
                pt_ps = work.tile([P, P], f16, tag="wk", name=f"ptps{t}_{kt}")
                nc.tensor.transpose(
                    pt_ps[:], p_sb[:, kt * P:(kt + 1) * P], ident[:]
                )
                pt_sb = ptpool.tile([P, P], f16, tag="pt", name=f"pt{t}_{kt}")
                nc.vector.tensor_copy(out=pt_sb[:], in_=pt_ps[:])
                pts.append(pt_sb)

            rinv = scal.tile([P, 1], f32, tag="rinv", name=f"rinv{t}")
            nc.vector.reciprocal(rinv[:], ell)

            # O(t) = P^T.T @ V accumulation over local key tiles
            o_ps = opsum.tile([P, D], f32, tag="o", name=f"ops{t}")
            for kt in range(L):
                for c in range(2):
                    nc.tensor.matmul(
                        o_ps[:, c * 512:(c + 1) * 512],
                        lhsT=pts[kt][:],
                        rhs=v_sb[:, kt, c * 512:(c + 1) * 512],
                        start=(kt == 0),
                        stop=(kt == L - 1),
                    )

            o_sb = wpool.tile([P, D], f32, tag="w", name=f"osb{t}")
            nc.scalar.mul(o_sb[:], o_ps[:], rinv[:])
            nc.sync.dma_start(out=out_d[t * P:(t + 1) * P, :], in_=o_sb[:])

        nc.sync.dma_start(out=ml_d[:], in_=stats_sb[:])

    nc.compile()
    _PROG_CACHE["nc"] = nc
    return nc


def make_in_maps(x, Wq, Wk, Wv):
    """Host-side sharding: returns per-core input dicts (core c = 2*b + h)."""
    x = np.asarray(x, dtype=np.float32)
    wq16 = np.asarray(Wq, dtype=np.float32).astype(np.float16)
    wk16 = np.asarray(Wk, dtype=np.float32).astype(np.float16)
    wv16 = np.asarray(Wv, dtype=np.float32).astype(np.float16)

    tri = np.where(
        np.arange(P)[None, :] <= np.arange(P)[:, None], 0.0, MASK_NEG
    ).astype(np.float32)
    full = np.full((P, P), MASK_NEG, dtype=np.float32)
    zero = np.zeros((P, P), dtype=np.float32)
    # mask[parity]: additive mask for the last local key tile of query
    # tile t (parity = t%2). Local tile u = 2*(t//2) + h:
    #   h=0: t even -> u==t (diagonal tri); t odd -> u==t-1 (visible)
    #   h=1: t even -> u==t+1 (fully masked); t odd -> u==t (diagonal tri)
    masks = [
        np.stack([tri, zero]),   # h = 0
        np.stack([full, tri]),   # h = 1
    ]

    in_maps = []
    for b in range(NB):
        xt = np.ascontiguousarray(x[b].T)          # [D, SEQ] f32
        xt16 = xt.astype(np.float16)
        xq = np.ascontiguousarray((xt * SCALE).astype(np.float16))
        for h in range(2):
            kcols = np.concatenate(
                [np.arange((2 * m + h) * P, (2 * m + h + 1) * P)
                 for m in range(HT)]
            )
            in_maps.append({
                "xk": np.ascontiguousarray(xt16[:, kcols]),
                "xq": xq,
                "wq": wq16,
                "wk": wk16,
                "wv": wv16,
                "mask": masks[h],
            })
    return in_maps


def assemble_output(results):
    """Log-sum-exp combine of the two partial softmax halves per batch."""
    out = np.empty((NB, SEQ, D), dtype=np.float32)
    for b in range(NB):
        r0 = results[2 * b]
        r1 = results[2 * b + 1]
        o0 = r0["out"].astype(np.float64)
        o1 = r1["out"].astype(np.float64)
        # ml is [P, 2*QT]: col 2t = -max, col 2t+1 = sum; q = t*128 + p
        ml0 = r0["ml"].astype(np.float64)
        ml1 = r1["ml"].astype(np.float64)
        m0 = -(ml0[:, 0::2].T.reshape(SEQ))
        l0 = ml0[:, 1::2].T.reshape(SEQ)
        m1 = -(ml1[:, 0::2].T.reshape(SEQ))
        l1 = ml1[:, 1::2].T.reshape(SEQ)
        mm = np.maximum(m0, m1)
        w0 = l0 * np.exp(m0 - mm)
        w1 = l1 * np.exp(m1 - mm)
        tot = w0 + w1
        w0 /= tot
        w1 /= tot
        out[b] = (o0 * w0[:, None] + o1 * w1[:, None]).astype(np.float32)
    return out


def run(inputs, trace=False, tmpdir=None):
    """Build, run on 8 cores, gather. Returns (output, BassKernelResults)."""
    _install_ntff_hook()
    from concourse.bass_utils import run_bass_kernel_spmd

    nc = build_program()
    in_maps = make_in_maps(
        inputs["x"], inputs["Wq"], inputs["Wk"], inputs["Wv"]
    )
    kw = {}
    if trace:
        kw["trace"] = True
        if tmpdir is not None:
            kw["tmpdir"] = tmpdir
    res = run_bass_kernel_spmd(nc, in_maps, list(range(8)), **kw)
    return assemble_output(res.results), res


def kernel(**inputs):
    out, _ = run(inputs, trace=False)
    return out
